# revision 52
# baseline (speedup 1.0000x reference)
"""Trainium2 Bass kernel for nn_AttnBlock: dynamic-filter correlation.

Math (per sample b):
  p1[l, :]  = 11x11x64 patch of im1 at position l (l over 30x30)
  scores[p, l] = <im2 patch at p, p1[l] / max(||p1[l]||, 1e-4)>
  out[p] = max_l scores[p, l]

Decomposition on device (per core = one (sample, p-half) pair):
  scores_un[p, l] = sum_{dy,dx} sum_c im2[c, p+(dy,dx)] * im1[c, l+(dy,dx)]
computed as 121 shift-matmuls (contraction over channels) accumulated in
PSUM.  Operand tiles are built on the HOST (layout + dtype staging,
zero FLOPs): partitions 0..63 hold the image, partitions 64..127 the
same shifted one more column (or row for the dx=10 tile), so each
K=128 plane covers two (dy,dx) shifts.  The stationary (im2) side uses
per-dx compact tiles; the moving (im1) side for dx<10 slices straight
out of one RAW pitch-40 image via [15 rows @40, 30 @1] APs (cuts DMA
bytes ~25% at ~20ns/matmul AP-step cost on a third of the matmuls —
net faster because the input stream stops stalling and the PE clock
never drops mid-stream).

Mixed precision: 34 of the 60.5 planes (dx 0..3 all dy; dx 4,5 dy<8;
dx 10 dy<8) are fp8 e4m3 packed TWO planes per matmul with DoubleRow
perf mode (2x the bf16 FLOP rate; measured 190ns per 450-col matmul
either way — NOT the cost model's 0.5 cyc/row), the rest stay bf16.
This lands rel_err ~1.8e-2 against the 2e-2 gate (measured: all-bf16
1.5e-3, all-fp8 2.2e-2; max-err scales ~sqrt(fp8 fraction) but lumpy,
so the split was chosen by direct measurement).  The fp8 stationary
tiles use a padded row pitch of 32 so every DoubleRow plane-pair
stride is a multiple of 16 bytes (s3_lw_dual_fp8 ISA restriction:
pair dim count 2, stride % 16 == 0; overlapping strided APs are
otherwise fine); output positions are enumerated in this padded space
everywhere (the bf16 stationary tiles are padded the same way; dead
px=30,31 columns are zero and dropped on assembly).

Norms: separable 11x11 box sum of im1^2 as an fp16 shift-add log tree
on DVE on a row-split [128, 25, 40] layout covering both l-halves at
once, channel sum via one f16 ones-matmul per half, then max -> sqrt
(scalar ACT) -> DVE reciprocal -> f16 rank-1 broadcast matmul to
[128, 900].  (Rsqrt/Reciprocal ACT are blocked by bass for accuracy.)

Per-chunk epilogues are split per l-half so the j0 half runs on DVE
while the PE streams j1.  A burst of dummy matmuls during the
input-DMA wait trips the PE HAM activity window so the real matmuls
start at 2.4 GHz.  Input DMA is sliced and kicked in consumption-
deadline order (fp8 bi0 tiles first) so the stream never stalls.

Sharding: 8 cores = 4 samples x 2 halves of the output-row dim (pure
data parallel, no cross-core communication).
"""

import sys

import numpy as np
import ml_dtypes

if "/opt/trn_rl_repo" not in sys.path:
    sys.path.insert(0, "/opt/trn_rl_repo")

B = 4
C = 64
H = W = 40
KER = 11
HP = WP = H - KER + 1  # 30
HALF = HP // 2  # 15 output rows per core
N_CORES = 2 * B
IM2_ROWS = HALF + KER - 1  # 25 input rows needed per half

NL = HALF * WP  # 450 l-columns per half
DX_BASES = [0, 2, 4, 6, 8, 10]
TILE_M = H * WP  # 1200: full-height moving tile serves both l-halves
WPAD = 32
TILE_FW = IM2_ROWS * WPAD  # 800 free elements per padded stationary tile
# padded output-position chunks (M = stationary free dim / PSUM partitions)
P_CHUNKS = [(0, 128), (128, 128), (256, 128), (384, 96)]
N_WARMUP = 8

# fp8 plane set S: bi 0,1 all dy; bi 2 dy 0..7; bi 5 dy 0..7 (34 planes
# = 68 shifts; measured rel_err 1.8e-2 vs the 2e-2 gate).
#
# Moving operands for dx<10 planes come straight out of the RAW image
# tensors (partitions 0..63 = im1, 64..127 = im1 shifted one column;
# pitch 40), sliced as a [pair, 15 rows @40, 30 cols @1] AP — the
# per-dx compact tiles are pure layout duplication the moving side
# never needed (no ISA restrictions there).  Only the dx=10 tile (row-
# shifted hi partitions) keeps a compact [128, 40, 30] tile.  l-half j
# adds 15 rows to the row offset.
#
# FP8_PAIRS: (stationary offset, stat stride, moving tensor 'raw'/'m5',
# moving offset, mov pair stride).
FP8_PAIRS = (
    [
        (TILE_FW * bi + WPAD * dy, WPAD, "raw", W * dy + 2 * bi, W)
        for bi in range(2)
        for dy in (0, 2, 4, 6, 8)
    ]
    + [(WPAD * 10, TILE_FW, "raw", W * 10, 2)]  # (0,10)+(1,10): bi stride 2
    + [
        (TILE_FW * 2 + WPAD * dy, WPAD, "raw", W * dy + 4, W)
        for dy in (0, 2, 4, 6)
    ]
    + [
        (TILE_FW * 3 + WPAD * dy, 2 * WPAD, "m5", WP * dy, 2 * WP)
        for dy in (0, 4)  # bi5 (dx=10) planes (0,2) and (4,6)
    ]
)
# bf16 planes: (stationary offset, moving tensor, moving offset, K)
BF16_PLANES = (
    [(WPAD * dy, "raw", W * dy + 4, 128) for dy in (8, 9, 10)]  # bi2
    + [(TILE_FW + WPAD * dy, "raw", W * dy + 6, 128) for dy in range(11)]  # bi3
    + [(2 * TILE_FW + WPAD * dy, "raw", W * dy + 8, 128) for dy in range(11)]  # bi4
    + [(3 * TILE_FW + WPAD * 8, "m5", WP * 8, 128)]  # bi5 dy=8
    + [(3 * TILE_FW + WPAD * 10, "m5", WP * 10, C)]  # (10,10), K=64, last
)

_PROGRAM = None


def _build_program():
    import concourse.bass as bass
    import concourse.tile as tile
    from concourse import bacc
    from concourse.ap import AP

    mybir = bass.mybir
    dt = mybir.dt
    f32 = dt.float32
    f16 = dt.float16
    bf16 = dt.bfloat16
    f8 = dt.float8e4
    DR = mybir.MatmulPerfMode.DoubleRow
    from contextlib import ExitStack

    nc = bacc.Bacc(
        "TRN2",
        target_bir_lowering=False,
        debug=False,
        enable_asserts=False,
        num_devices=N_CORES,
    )
    w8_d = nc.dram_tensor("w8", [128, 4 * TILE_FW], f8, kind="ExternalInput").ap()
    raw8_d = nc.dram_tensor("raw8", [128, H * W], f8, kind="ExternalInput").ap()
    m58_d = nc.dram_tensor("m58", [128, H * WP], f8, kind="ExternalInput").ap()
    w16_d = nc.dram_tensor("w16", [128, 4 * TILE_FW], bf16, kind="ExternalInput").ap()
    raw16_d = nc.dram_tensor("raw16", [128, H * W], bf16, kind="ExternalInput").ap()
    m516_d = nc.dram_tensor("m516", [128, H * WP], bf16, kind="ExternalInput").ap()
    im1n_d = nc.dram_tensor("im1n", [128, IM2_ROWS, W], bf16, kind="ExternalInput").ap()
    out_d = nc.dram_tensor("out", [128, 4], f32, kind="ExternalOutput").ap()

    MULT = mybir.AluOpType.mult
    MAX = mybir.AluOpType.max
    SQRT = mybir.ActivationFunctionType.Sqrt

    with tile.TileContext(nc) as tc, ExitStack() as ctx:
        consts = ctx.enter_context(tc.tile_pool(name="consts", bufs=1))
        imgs = ctx.enter_context(tc.tile_pool(name="imgs", bufs=1))
        nrm = ctx.enter_context(tc.tile_pool(name="nrm", bufs=1))
        scr = ctx.enter_context(tc.tile_pool(name="scr", bufs=2))
        reds = ctx.enter_context(tc.tile_pool(name="reds", bufs=8))
        psum = ctx.enter_context(tc.tile_pool(name="psum", bufs=8, space="PSUM"))

        w8 = imgs.tile([128, 4 * TILE_FW], f8, name="w8")
        raw8 = imgs.tile([128, H * W], f8, name="raw8")
        m58 = imgs.tile([128, H * WP], f8, name="m58")
        w16 = imgs.tile([128, 4 * TILE_FW], bf16, name="w16")
        raw16 = imgs.tile([128, H * W], bf16, name="raw16")
        m516 = imgs.tile([128, H * WP], bf16, name="m516")
        im1n = imgs.tile([128, IM2_ROWS, W], bf16)

        def pair_ap(t, off, step, length, kp=128):
            base = t[:]
            pstride = base.ap[0][0]
            return AP(base.tensor, base.offset + off, [[pstride, kp], [step, 2], [1, length]])

        def raw_rhs(t, off, j, kp=128, step=None):
            # moving slice out of a raw pitch-40 image: 15 rows x 30 cols,
            # optionally with a leading DoubleRow pair dim of stride `step`.
            base = t[:]
            pstride = base.ap[0][0]
            dims = [[pstride, kp]]
            if step is not None:
                dims.append([step, 2])
            dims += [[W, HALF], [1, WP]]
            return AP(base.tensor, base.offset + off + HALF * W * j, dims)

        # Warm-up consts on vector (no DMA kicks there), so the PE warm-up
        # matmuls can start as soon as the framework preamble retires.
        warm_lhs = consts.tile([128, 128], f16)
        nc.vector.memset(warm_lhs[:], 1.0)
        warm_rhs = consts.tile([128, 512], f16)
        nc.vector.memset(warm_rhs[:], 0.25)
        ones_row = consts.tile([1, 128], f16)
        nc.vector.memset(ones_row[:], 1.0)
        ones_col = consts.tile([128, 1], f16)
        nc.vector.memset(ones_col[:], 1.0)
        red_all = reds.tile([128, 4], f32, name="red_all")
        nc.vector.memset(red_all[:], 0.0)

        # DMA kicks in consumption-deadline order (the DMA engines
        # processor-share bandwidth, so this ordering is best-effort; all
        # kicks go up front — measured faster than completion-chained
        # staggering).  bi2's bf16 slices are trimmed to the columns/rows
        # its dy>=8 planes actually use.
        # The DMA engines processor-share bandwidth across outstanding
        # kicks, and SMALL kicks exit the share pool first — so the
        # stream-critical fp8 tensors are shredded into ~0.1MB kicks
        # (they complete before the stream starts), while the later-
        # needed bf16 tensors stay as big kicks (PS naturally finishes
        # them last, roughly in deadline order).
        nc.sync.dma_start(raw8[:, 0:800], raw8_d[:, 0:800])
        nc.scalar.dma_start(w8[:, 0:TILE_FW], w8_d[:, 0:TILE_FW])
        nc.gpsimd.dma_start(raw8[:, 800:1600], raw8_d[:, 800:1600])
        nc.sync.dma_start(w8[:, TILE_FW : 2 * TILE_FW], w8_d[:, TILE_FW : 2 * TILE_FW])
        nc.scalar.dma_start(m58[:], m58_d)
        nc.gpsimd.dma_start(w8[:, 2 * TILE_FW : 3 * TILE_FW], w8_d[:, 2 * TILE_FW : 3 * TILE_FW])
        nc.sync.dma_start(w8[:, 3 * TILE_FW : 4 * TILE_FW], w8_d[:, 3 * TILE_FW : 4 * TILE_FW])
        nc.scalar.dma_start(raw16[:], raw16_d)
        nc.gpsimd.dma_start(w16[:, 8 * WPAD : 2 * TILE_FW], w16_d[:, 8 * WPAD : 2 * TILE_FW])
        nc.sync.dma_start(w16[:, 2 * TILE_FW : 4 * TILE_FW], w16_d[:, 2 * TILE_FW : 4 * TILE_FW])
        nc.scalar.dma_start(m516[:], m516_d)
        nc.gpsimd.dma_start(im1n[:], im1n_d)

        # ---- PE warm-up: trip the HAM activity window during the DMA wait
        # so the real matmuls start at 2.4 GHz.  Results are never read.
        wps = psum.tile([128, 512], f32, tag="ps", name="warm")
        for i in range(N_WARMUP):
            nc.tensor.matmul(wps[:], warm_lhs[:], warm_rhs[:], start=True, stop=True)

        # ---- norm DVE chain: separable 11x11 box sum of im1^2 over (y, x)
        # on the row-split layout (partitions 0..63 = rows 0..24 -> l-half 0,
        # partitions 64..127 = rows 15..39 -> l-half 1), all in fp16.
        sq = nrm.tile([128, IM2_ROWS, W], f16)
        nc.vector.tensor_tensor(out=sq[:], in0=im1n[:], in1=im1n[:], op=MULT)
        t2 = nrm.tile([128, IM2_ROWS, W - 1], f16)
        nc.vector.tensor_add(t2[:], sq[:, :, 0 : W - 1], sq[:, :, 1:W])
        t4 = nrm.tile([128, IM2_ROWS, W - 3], f16)
        nc.vector.tensor_add(t4[:], t2[:, :, 0 : W - 3], t2[:, :, 2 : W - 1])
        t8 = nrm.tile([128, IM2_ROWS, W - 7], f16)
        nc.vector.tensor_add(t8[:], t4[:, :, 0 : W - 7], t4[:, :, 4 : W - 3])
        rpa = nrm.tile([128, IM2_ROWS, WP], f16)
        nc.vector.tensor_add(rpa[:], t8[:, :, 0:WP], t2[:, :, 8 : 8 + WP])
        rp = nrm.tile([128, IM2_ROWS, WP], f16)
        nc.vector.tensor_add(rp[:], rpa[:], sq[:, :, 10 : 10 + WP])

        u2 = nrm.tile([128, IM2_ROWS - 1, WP], f16)
        nc.vector.tensor_add(u2[:], rp[:, 0 : IM2_ROWS - 1], rp[:, 1:IM2_ROWS])
        u4 = nrm.tile([128, IM2_ROWS - 3, WP], f16)
        nc.vector.tensor_add(u4[:], u2[:, 0 : IM2_ROWS - 3], u2[:, 2 : IM2_ROWS - 1])
        u8 = nrm.tile([128, IM2_ROWS - 7, WP], f16)
        nc.vector.tensor_add(u8[:], u4[:, 0 : IM2_ROWS - 7], u4[:, 4 : IM2_ROWS - 3])
        nca = nrm.tile([128, HALF, WP], f16)
        nc.vector.tensor_add(nca[:], u8[:, 0:HALF], u2[:, 8 : 8 + HALF])
        normc = nrm.tile([128, HALF, WP], f16)
        nc.vector.tensor_add(normc[:], nca[:], rp[:, 10 : 10 + HALF])
        normc_f = normc[:].rearrange("p y x -> p (y x)")

        # ---- main correlation matmuls: 15 fp8 DoubleRow + 31 bf16 per
        # (chunk, l-half).  Padded-p chunks, M<=128 stationary.  The fp8
        # sub-stream is emitted for both l-halves before the bf16 one so
        # the (larger, later-arriving) bf16 tiles get extra DMA headroom.
        def emit_chunk_j8(p0, M, j, ps_j):
            first = True
            for woff, wstep, mt, moff, mstep in FP8_PAIRS:
                lhsT = pair_ap(w8, woff + p0, wstep, M)
                if mt == "raw":
                    rhs = raw_rhs(raw8, moff, j, step=mstep)
                else:
                    rhs = pair_ap(m58, moff + NL * j, mstep, NL)
                nc.tensor.matmul(
                    ps_j[0:M], lhsT, rhs, start=first, stop=False, perf_mode=DR
                )
                first = False

        def emit_chunk_j16(p0, M, j, ps_j):
            for woff, mt, moff, kp in BF16_PLANES:
                lhsT = w16[0:kp, woff + p0 : woff + p0 + M]
                if mt == "raw":
                    rhs = raw_rhs(raw16, moff, j, kp=kp)
                else:
                    rhs = m516[0:kp, moff + NL * j : moff + NL * j + NL]
                last = kp == C
                nc.tensor.matmul(ps_j[0:M], lhsT, rhs, start=False, stop=last)

        def alloc_ps(ci):
            return [
                psum.tile([128, NL], f32, tag="ps", name=f"ps_{ci}_{j}")
                for j in range(2)
            ]

        def emit_epi_j0(ci, M, ps):
            sc0 = scr.tile([128, NL], f32, tag="sc", name=f"sc0_{ci}")
            red0 = reds.tile([128, 1], f32, tag="red", name=f"red0_{ci}")
            nc.vector.tensor_tensor(
                out=sc0[0:M], in0=ps[0][0:M], in1=inv_bc[0:M, 0:NL], op=MULT
            )
            nc.vector.tensor_reduce(
                out=red0[0:M], in_=sc0[0:M], axis=mybir.AxisListType.X, op=MAX
            )
            return red0

        def emit_epi_j1(ci, M, ps, red0):
            sc1 = scr.tile([128, NL], f32, tag="sc", name=f"sc1_{ci}")
            red1 = reds.tile([128, 1], f32, tag="red2", name=f"red1_{ci}")
            nc.vector.tensor_tensor(
                out=sc1[0:M], in0=ps[1][0:M], in1=inv_bc[0:M, NL : 2 * NL], op=MULT
            )
            nc.vector.tensor_reduce(
                out=red1[0:M], in_=sc1[0:M], axis=mybir.AxisListType.X, op=MAX
            )
            nc.vector.tensor_tensor(
                out=red_all[0:M, ci : ci + 1], in0=red0[0:M], in1=red1[0:M], op=MAX
            )

        def emit_chunk_j(p0, M, j, ps_j):
            emit_chunk_j8(p0, M, j, ps_j)
            emit_chunk_j16(p0, M, j, ps_j)

        chunk_ps = {}
        chunk_red = {}

        # chunk 0, both l-halves
        chunk_ps[0] = alloc_ps(0)
        emit_chunk_j(*P_CHUNKS[0], 0, chunk_ps[0][0])
        emit_chunk_j(*P_CHUNKS[0], 1, chunk_ps[0][1])

        # norm matmuls: f16 ones channel-sum per l-half, placed after chunk
        # 0 so the fp16 tree has a wide completion margin.
        nm = [psum.tile([1, NL], f32, tag="ps", name=f"nm_{j}") for j in range(2)]
        nc.tensor.matmul(nm[0][:], ones_col[0:C, :], normc_f[0:C, :], start=True, stop=True)
        nc.tensor.matmul(nm[1][:], ones_col[C : 2 * C, :], normc_f[C : 2 * C, :], start=True, stop=True)

        nsq = nrm.tile([1, 2 * NL], f32)
        nc.vector.tensor_scalar_max(nsq[:, 0:NL], nm[0][:], 1e-8)
        nc.vector.tensor_scalar_max(nsq[:, NL : 2 * NL], nm[1][:], 1e-8)
        nrm_s = nrm.tile([1, 2 * NL], f32)
        nc.scalar.activation(nrm_s[:], nsq[:], SQRT)
        inv_s = nrm.tile([1, 2 * NL], f32)
        nc.vector.reciprocal(inv_s[:], nrm_s[:])
        inv16 = nrm.tile([1, 2 * NL], f16)
        nc.vector.tensor_copy(inv16[:], inv_s[:])

        # chunk 1 first half
        chunk_ps[1] = alloc_ps(1)
        emit_chunk_j(*P_CHUNKS[1], 0, chunk_ps[1][0])

        # rank-1 broadcast of 1/norm to all 128 partitions, between chunk
        # 1's halves (inv16 is ready by the time the PE arrives here).
        inv_bc = nrm.tile([128, 2 * NL], f32)
        for j in range(2):
            ip = psum.tile([128, NL], f32, tag="ps", name=f"ip_{j}")
            nc.tensor.matmul(ip[:], ones_row[:], inv16[:, NL * j : NL * (j + 1)], start=True, stop=True)
            nc.vector.tensor_copy(inv_bc[:, NL * j : NL * (j + 1)], ip[:])

        emit_chunk_j(*P_CHUNKS[1], 1, chunk_ps[1][1])

        chunk_red[0] = emit_epi_j0(0, P_CHUNKS[0][1], chunk_ps[0])
        chunk_ps[2] = alloc_ps(2)
        emit_chunk_j(*P_CHUNKS[2], 0, chunk_ps[2][0])
        emit_epi_j1(0, P_CHUNKS[0][1], chunk_ps[0], chunk_red[0])
        nc.scalar.dma_start(out_d[:, 0:1], red_all[:, 0:1])
        chunk_red[1] = emit_epi_j0(1, P_CHUNKS[1][1], chunk_ps[1])
        emit_chunk_j(*P_CHUNKS[2], 1, chunk_ps[2][1])
        emit_epi_j1(1, P_CHUNKS[1][1], chunk_ps[1], chunk_red[1])
        nc.scalar.dma_start(out_d[:, 1:2], red_all[:, 1:2])
        chunk_ps[3] = alloc_ps(3)
        emit_chunk_j(*P_CHUNKS[3], 0, chunk_ps[3][0])
        chunk_red[2] = emit_epi_j0(2, P_CHUNKS[2][1], chunk_ps[2])
        emit_chunk_j(*P_CHUNKS[3], 1, chunk_ps[3][1])
        emit_epi_j1(2, P_CHUNKS[2][1], chunk_ps[2], chunk_red[2])
        nc.scalar.dma_start(out_d[:, 2:3], red_all[:, 2:3])
        chunk_red[3] = emit_epi_j0(3, P_CHUNKS[3][1], chunk_ps[3])
        emit_epi_j1(3, P_CHUNKS[3][1], chunk_ps[3], chunk_red[3])
        nc.scalar.dma_start(out_d[:, 3:4], red_all[:, 3:4])

    nc.compile()
    return nc


def _get_program():
    global _PROGRAM
    if _PROGRAM is None:
        _PROGRAM = _build_program()
    return _PROGRAM


F8 = ml_dtypes.float8_e4m3
BF16 = ml_dtypes.bfloat16


def _pad32(t):
    """[128, R, 30] -> [128, R*32] f32, rows padded 30->32 with 0."""
    padded = np.zeros((128, IM2_ROWS, WPAD), np.float32)
    padded[:, :, :WP] = t
    return padded.reshape(128, -1)


def make_in_maps(im1: np.ndarray, im2: np.ndarray):
    im1 = np.asarray(im1, dtype=np.float32)
    im2 = np.asarray(im2, dtype=np.float32)
    in_maps = []
    for b in range(B):
        i1 = im1[b]
        i1pad = np.concatenate([i1, np.zeros((C, 1, W), np.float32)], axis=1)
        i1s = i1pad[:, 1 : H + 1, :]  # rows shifted down one, zero row at end
        i1x = np.concatenate(
            [i1[:, :, 1:], np.zeros((C, H, 1), np.float32)], axis=2
        )  # cols shifted left one (hi-partition half of the raw tensors)
        raw = np.concatenate([i1, i1x], axis=0).reshape(128, -1)
        m5 = np.concatenate([i1[:, :, 10:40], i1s[:, :, 10:40]], axis=0).reshape(128, -1)
        raw8 = np.ascontiguousarray(raw.astype(F8))
        raw16 = np.ascontiguousarray(raw.astype(BF16))
        m58 = np.ascontiguousarray(m5.astype(F8))
        m516 = np.ascontiguousarray(m5.astype(BF16))
        im1n = np.ascontiguousarray(
            np.concatenate(
                [i1[:, 0:IM2_ROWS, :], i1[:, HALF : HALF + IM2_ROWS, :]], axis=0
            ).astype(BF16)
        )
        for h in range(2):
            y0 = HALF * h
            i2 = im2[b][:, y0 : y0 + IM2_ROWS, :]
            i2pad = np.concatenate(
                [im2[b], np.zeros((C, 1, W), np.float32)], axis=1
            )[:, y0 + 1 : y0 + 1 + IM2_ROWS, :]
            wt = []
            for bi, dx in enumerate(DX_BASES):
                if dx < 10:
                    wt.append(
                        _pad32(
                            np.concatenate(
                                [i2[:, :, dx : dx + WP], i2[:, :, dx + 1 : dx + WP + 1]],
                                axis=0,
                            )
                        )
                    )
                else:
                    wt.append(
                        _pad32(np.concatenate([i2[:, :, 10:40], i2pad[:, :, 10:40]], axis=0))
                    )
            m = {
                "raw8": raw8,
                "raw16": raw16,
                "m58": m58,
                "m516": m516,
                "im1n": im1n,
                "w8": np.ascontiguousarray(
                    np.concatenate([wt[i] for i in (0, 1, 2, 5)], axis=1).astype(F8)
                ),
                "w16": np.ascontiguousarray(np.concatenate(wt[2:6], axis=1).astype(BF16)),
            }
            in_maps.append(m)
    return in_maps


def _half_from_cols(cols):
    flat = np.empty((HALF * WPAD,), dtype=np.float32)
    for ci, (p0, M) in enumerate(P_CHUNKS):
        flat[p0 : p0 + M] = cols[0:M, ci]
    return flat.reshape(HALF, WPAD)[:, :WP]


def assemble(results):
    out = np.empty((B, 1, HP, WP), dtype=np.float32)
    for b in range(B):
        top = _half_from_cols(results[2 * b]["out"])
        bot = _half_from_cols(results[2 * b + 1]["out"])
        out[b, 0] = np.concatenate([top, bot], axis=0)
    return out


def run(im1: np.ndarray, im2: np.ndarray, trace: bool = False):
    from concourse import bass_utils

    nc = _get_program()
    res = bass_utils.run_bass_kernel_spmd(
        nc, make_in_maps(im1, im2), core_ids=list(range(N_CORES)), trace=trace
    )
    return assemble(res.results), res


def kernel(im1: np.ndarray, im2: np.ndarray) -> np.ndarray:
    out, _ = run(np.asarray(im1), np.asarray(im2))
    return out


# revision 53
# speedup vs baseline: 1.1883x; 1.1883x over previous
"""Trainium2 Bass kernel for nn_AttnBlock: dynamic-filter correlation.

Math (per sample b):
  p1[l, :]  = 11x11x64 patch of im1 at position l (l over 30x30)
  scores[p, l] = <im2 patch at p, p1[l] / max(||p1[l]||, 1e-4)>
  out[p] = max_l scores[p, l]

Decomposition on device (per core = one (sample, p-half) pair):
  scores_un[p, l] = sum_{dy,dx} sum_c im2[c, p+(dy,dx)] * im1[c, l+(dy,dx)]
computed as 121 shift-matmuls (contraction over channels) accumulated in
PSUM.  Operand tiles are built on the HOST (layout + dtype staging,
zero FLOPs): partitions 0..63 hold the image, partitions 64..127 the
same shifted one more column (or row for the dx=10 tile), so each
K=128 plane covers two (dy,dx) shifts.  The stationary (im2) side uses
per-dx compact tiles; the moving (im1) side for dx<10 slices straight
out of one RAW pitch-40 image via [15 rows @40, 30 @1] APs (cuts DMA
bytes ~25% at ~20ns/matmul AP-step cost on a third of the matmuls —
net faster because the input stream stops stalling and the PE clock
never drops mid-stream).

Mixed precision: 34 of the 60.5 planes (dx 0..3 all dy; dx 4,5 dy<8;
dx 10 dy<8) are fp8 e4m3 packed TWO planes per matmul with DoubleRow
perf mode (2x the bf16 FLOP rate; measured 190ns per 450-col matmul
either way — NOT the cost model's 0.5 cyc/row), the rest stay bf16.
This lands rel_err ~1.8e-2 against the 2e-2 gate (measured: all-bf16
1.5e-3, all-fp8 2.2e-2; max-err scales ~sqrt(fp8 fraction) but lumpy,
so the split was chosen by direct measurement).  The fp8 stationary
tiles use a padded row pitch of 32 so every DoubleRow plane-pair
stride is a multiple of 16 bytes (s3_lw_dual_fp8 ISA restriction:
pair dim count 2, stride % 16 == 0; overlapping strided APs are
otherwise fine); output positions are enumerated in this padded space
everywhere (the bf16 stationary tiles are padded the same way; dead
px=30,31 columns are zero and dropped on assembly).

Norms: separable 11x11 box sum of im1^2 as an fp16 shift-add log tree
on DVE on a row-split [128, 25, 40] layout covering both l-halves at
once, channel sum via one f16 ones-matmul per half, then max -> sqrt
(scalar ACT) -> DVE reciprocal -> f16 rank-1 broadcast matmul to
[128, 900].  (Rsqrt/Reciprocal ACT are blocked by bass for accuracy.)

Per-chunk epilogues are split per l-half so the j0 half runs on DVE
while the PE streams j1.  A burst of dummy matmuls during the
input-DMA wait trips the PE HAM activity window so the real matmuls
start at 2.4 GHz.  Input DMA is sliced and kicked in consumption-
deadline order (fp8 bi0 tiles first) so the stream never stalls.

Sharding: 8 cores = 4 samples x 2 halves of the output-row dim (pure
data parallel, no cross-core communication).
"""

import sys

import numpy as np
import ml_dtypes

if "/opt/trn_rl_repo" not in sys.path:
    sys.path.insert(0, "/opt/trn_rl_repo")

B = 4
C = 64
H = W = 40
KER = 11
HP = WP = H - KER + 1  # 30
HALF = HP // 2  # 15 output rows per core
N_CORES = 2 * B
IM2_ROWS = HALF + KER - 1  # 25 input rows needed per half

NL = HALF * WP  # 450 l-columns per half
DX_BASES = [0, 2, 4, 6, 8, 10]
TILE_M = H * WP  # 1200: full-height moving tile serves both l-halves
WPAD = 32
TILE_FW = IM2_ROWS * WPAD  # 800 free elements per padded stationary tile
# padded output-position chunks (M = stationary free dim / PSUM partitions)
P_CHUNKS = [(0, 128), (128, 128), (256, 128), (384, 96)]
N_WARMUP = 7

# fp8 plane set S: bi 0,1 all dy; bi 2 dy 0..7; bi 5 dy 0..7 (34 planes
# = 68 shifts; measured rel_err 1.8e-2 vs the 2e-2 gate).
#
# Moving operands for dx<10 planes come straight out of the RAW image
# tensors (partitions 0..63 = im1, 64..127 = im1 shifted one column;
# pitch 40), sliced as a [pair, 15 rows @40, 30 cols @1] AP — the
# per-dx compact tiles are pure layout duplication the moving side
# never needed (no ISA restrictions there).  Only the dx=10 tile (row-
# shifted hi partitions) keeps a compact [128, 40, 30] tile.  l-half j
# adds 15 rows to the row offset.
#
# FP8_PAIRS: (stationary offset, stat stride, moving tensor 'raw'/'m5',
# moving offset, mov pair stride).
FP8_PAIRS = (
    [
        (TILE_FW * bi + WPAD * dy, WPAD, "raw", W * dy + 2 * bi, W)
        for bi in range(2)
        for dy in (0, 2, 4, 6, 8)
    ]
    + [(WPAD * 10, TILE_FW, "raw", W * 10, 2)]  # (0,10)+(1,10): bi stride 2
    + [
        (TILE_FW * 2 + WPAD * dy, WPAD, "raw", W * dy + 4, W)
        for dy in (0, 2, 4, 6)
    ]
    + [
        (TILE_FW * 3 + WPAD * dy, 2 * WPAD, "m5", WP * dy, 2 * WP)
        for dy in (0, 4)  # bi5 (dx=10) planes (0,2) and (4,6)
    ]
)
# bf16 planes: (stationary offset, moving tensor, moving offset, K)
BF16_PLANES = (
    [(WPAD * dy, "raw", W * dy + 4, 128) for dy in (8, 9, 10)]  # bi2
    + [(TILE_FW + WPAD * dy, "raw", W * dy + 6, 128) for dy in range(11)]  # bi3
    + [(2 * TILE_FW + WPAD * dy, "raw", W * dy + 8, 128) for dy in range(11)]  # bi4
    + [(3 * TILE_FW + WPAD * 8, "m5", WP * 8, 128)]  # bi5 dy=8
    + [(3 * TILE_FW + WPAD * 10, "m5", WP * 10, C)]  # (10,10), K=64, last
)

_PROGRAM = None


def _build_program():
    import concourse.bass as bass
    import concourse.tile as tile
    from concourse import bacc
    from concourse.ap import AP

    mybir = bass.mybir
    dt = mybir.dt
    f32 = dt.float32
    f16 = dt.float16
    bf16 = dt.bfloat16
    f8 = dt.float8e4
    DR = mybir.MatmulPerfMode.DoubleRow
    from contextlib import ExitStack

    nc = bacc.Bacc(
        "TRN2",
        target_bir_lowering=False,
        debug=False,
        enable_asserts=False,
        num_devices=N_CORES,
    )
    w8_d = nc.dram_tensor("w8", [128, 4 * TILE_FW], f8, kind="ExternalInput").ap()
    raw8_d = nc.dram_tensor("raw8", [128, H * W], f8, kind="ExternalInput").ap()
    m58_d = nc.dram_tensor("m58", [128, H * WP], f8, kind="ExternalInput").ap()
    w16_d = nc.dram_tensor("w16", [128, 4 * TILE_FW], bf16, kind="ExternalInput").ap()
    raw16_d = nc.dram_tensor("raw16", [128, H * W], bf16, kind="ExternalInput").ap()
    m516_d = nc.dram_tensor("m516", [128, H * WP], bf16, kind="ExternalInput").ap()
    im1n_d = nc.dram_tensor("im1n", [128, IM2_ROWS, W], bf16, kind="ExternalInput").ap()
    out_d = nc.dram_tensor("out", [128, 4], f32, kind="ExternalOutput").ap()

    MULT = mybir.AluOpType.mult
    MAX = mybir.AluOpType.max
    SQRT = mybir.ActivationFunctionType.Sqrt

    with tile.TileContext(nc) as tc, ExitStack() as ctx:
        consts = ctx.enter_context(tc.tile_pool(name="consts", bufs=1))
        imgs = ctx.enter_context(tc.tile_pool(name="imgs", bufs=1))
        nrm = ctx.enter_context(tc.tile_pool(name="nrm", bufs=1))
        scr = ctx.enter_context(tc.tile_pool(name="scr", bufs=2))
        reds = ctx.enter_context(tc.tile_pool(name="reds", bufs=8))
        psum = ctx.enter_context(tc.tile_pool(name="psum", bufs=8, space="PSUM"))

        w8 = imgs.tile([128, 4 * TILE_FW], f8, name="w8")
        raw8 = imgs.tile([128, H * W], f8, name="raw8")
        m58 = imgs.tile([128, H * WP], f8, name="m58")
        w16 = imgs.tile([128, 4 * TILE_FW], bf16, name="w16")
        raw16 = imgs.tile([128, H * W], bf16, name="raw16")
        m516 = imgs.tile([128, H * WP], bf16, name="m516")
        im1n = imgs.tile([128, IM2_ROWS, W], bf16)

        def pair_ap(t, off, step, length, kp=128):
            base = t[:]
            pstride = base.ap[0][0]
            return AP(base.tensor, base.offset + off, [[pstride, kp], [step, 2], [1, length]])

        def raw_rhs(t, off, j, kp=128, step=None):
            # moving slice out of a raw pitch-40 image: 15 rows x 30 cols,
            # optionally with a leading DoubleRow pair dim of stride `step`.
            base = t[:]
            pstride = base.ap[0][0]
            dims = [[pstride, kp]]
            if step is not None:
                dims.append([step, 2])
            dims += [[W, HALF], [1, WP]]
            return AP(base.tensor, base.offset + off + HALF * W * j, dims)

        # Warm-up consts on vector (no DMA kicks there), so the PE warm-up
        # matmuls can start as soon as the framework preamble retires.
        warm_lhs = consts.tile([128, 128], f16)
        nc.vector.memset(warm_lhs[:], 1.0)
        warm_rhs = consts.tile([128, 512], f16)
        nc.vector.memset(warm_rhs[:], 0.25)
        ones_row = consts.tile([1, 128], f16)
        nc.vector.memset(ones_row[:], 1.0)
        ones_col = consts.tile([128, 1], f16)
        nc.vector.memset(ones_col[:], 1.0)
        red_all = reds.tile([128, 4], f32, name="red_all")
        nc.vector.memset(red_all[:], 0.0)

        # DMA kicks in consumption-deadline order (the DMA engines
        # processor-share bandwidth, so this ordering is best-effort; all
        # kicks go up front — measured faster than completion-chained
        # staggering).  bi2's bf16 slices are trimmed to the columns/rows
        # its dy>=8 planes actually use.
        nc.sync.dma_start(raw8[:], raw8_d)
        nc.scalar.dma_start(w8[:], w8_d)
        nc.gpsimd.dma_start(m58[:], m58_d)
        nc.sync.dma_start(raw16[:], raw16_d)
        nc.scalar.dma_start(w16[:, 8 * WPAD : 2 * TILE_FW], w16_d[:, 8 * WPAD : 2 * TILE_FW])
        nc.gpsimd.dma_start(w16[:, 2 * TILE_FW : 4 * TILE_FW], w16_d[:, 2 * TILE_FW : 4 * TILE_FW])
        nc.sync.dma_start(m516[:], m516_d)
        nc.scalar.dma_start(im1n[:], im1n_d)

        # ---- PE warm-up: trip the HAM activity window during the DMA wait
        # so the real matmuls start at 2.4 GHz.  Results are never read.
        wps = psum.tile([128, 512], f32, tag="ps", name="warm")
        for i in range(N_WARMUP):
            nc.tensor.matmul(wps[:], warm_lhs[:], warm_rhs[:], start=True, stop=True)

        # ---- norm DVE chain: separable 11x11 box sum of im1^2 over (y, x)
        # on the row-split layout (partitions 0..63 = rows 0..24 -> l-half 0,
        # partitions 64..127 = rows 15..39 -> l-half 1), all in fp16.
        sq = nrm.tile([128, IM2_ROWS, W], f16)
        nc.vector.tensor_tensor(out=sq[:], in0=im1n[:], in1=im1n[:], op=MULT)
        t2 = nrm.tile([128, IM2_ROWS, W - 1], f16)
        nc.vector.tensor_add(t2[:], sq[:, :, 0 : W - 1], sq[:, :, 1:W])
        t4 = nrm.tile([128, IM2_ROWS, W - 3], f16)
        nc.vector.tensor_add(t4[:], t2[:, :, 0 : W - 3], t2[:, :, 2 : W - 1])
        t8 = nrm.tile([128, IM2_ROWS, W - 7], f16)
        nc.vector.tensor_add(t8[:], t4[:, :, 0 : W - 7], t4[:, :, 4 : W - 3])
        rpa = nrm.tile([128, IM2_ROWS, WP], f16)
        nc.vector.tensor_add(rpa[:], t8[:, :, 0:WP], t2[:, :, 8 : 8 + WP])
        rp = nrm.tile([128, IM2_ROWS, WP], f16)
        nc.vector.tensor_add(rp[:], rpa[:], sq[:, :, 10 : 10 + WP])

        u2 = nrm.tile([128, IM2_ROWS - 1, WP], f16)
        nc.vector.tensor_add(u2[:], rp[:, 0 : IM2_ROWS - 1], rp[:, 1:IM2_ROWS])
        u4 = nrm.tile([128, IM2_ROWS - 3, WP], f16)
        nc.vector.tensor_add(u4[:], u2[:, 0 : IM2_ROWS - 3], u2[:, 2 : IM2_ROWS - 1])
        u8 = nrm.tile([128, IM2_ROWS - 7, WP], f16)
        nc.vector.tensor_add(u8[:], u4[:, 0 : IM2_ROWS - 7], u4[:, 4 : IM2_ROWS - 3])
        nca = nrm.tile([128, HALF, WP], f16)
        nc.vector.tensor_add(nca[:], u8[:, 0:HALF], u2[:, 8 : 8 + HALF])
        normc = nrm.tile([128, HALF, WP], f16)
        nc.vector.tensor_add(normc[:], nca[:], rp[:, 10 : 10 + HALF])
        normc_f = normc[:].rearrange("p y x -> p (y x)")

        # ---- main correlation matmuls: 15 fp8 DoubleRow + 31 bf16 per
        # (chunk, l-half).  Padded-p chunks, M<=128 stationary.  The fp8
        # sub-stream is emitted for both l-halves before the bf16 one so
        # the (larger, later-arriving) bf16 tiles get extra DMA headroom.
        def emit_chunk_j8(p0, M, j, ps_j):
            first = True
            for woff, wstep, mt, moff, mstep in FP8_PAIRS:
                lhsT = pair_ap(w8, woff + p0, wstep, M)
                if mt == "raw":
                    rhs = raw_rhs(raw8, moff, j, step=mstep)
                else:
                    rhs = pair_ap(m58, moff + NL * j, mstep, NL)
                nc.tensor.matmul(
                    ps_j[0:M], lhsT, rhs, start=first, stop=False, perf_mode=DR
                )
                first = False

        def emit_chunk_j16(p0, M, j, ps_j):
            for woff, mt, moff, kp in BF16_PLANES:
                lhsT = w16[0:kp, woff + p0 : woff + p0 + M]
                if mt == "raw":
                    rhs = raw_rhs(raw16, moff, j, kp=kp)
                else:
                    rhs = m516[0:kp, moff + NL * j : moff + NL * j + NL]
                last = kp == C
                nc.tensor.matmul(ps_j[0:M], lhsT, rhs, start=False, stop=last)

        def alloc_ps(ci):
            return [
                psum.tile([128, NL], f32, tag="ps", name=f"ps_{ci}_{j}")
                for j in range(2)
            ]

        def emit_epi_j0(ci, M, ps):
            sc0 = scr.tile([128, NL], f32, tag="sc", name=f"sc0_{ci}")
            red0 = reds.tile([128, 1], f32, tag="red", name=f"red0_{ci}")
            nc.vector.tensor_tensor(
                out=sc0[0:M], in0=ps[0][0:M], in1=inv_bc[0:M, 0:NL], op=MULT
            )
            nc.vector.tensor_reduce(
                out=red0[0:M], in_=sc0[0:M], axis=mybir.AxisListType.X, op=MAX
            )
            return red0

        def emit_epi_j1(ci, M, ps, red0):
            sc1 = scr.tile([128, NL], f32, tag="sc", name=f"sc1_{ci}")
            red1 = reds.tile([128, 1], f32, tag="red2", name=f"red1_{ci}")
            nc.vector.tensor_tensor(
                out=sc1[0:M], in0=ps[1][0:M], in1=inv_bc[0:M, NL : 2 * NL], op=MULT
            )
            nc.vector.tensor_reduce(
                out=red1[0:M], in_=sc1[0:M], axis=mybir.AxisListType.X, op=MAX
            )
            nc.vector.tensor_tensor(
                out=red_all[0:M, ci : ci + 1], in0=red0[0:M], in1=red1[0:M], op=MAX
            )

        def emit_chunk_j(p0, M, j, ps_j):
            emit_chunk_j8(p0, M, j, ps_j)
            emit_chunk_j16(p0, M, j, ps_j)

        chunk_ps = {}
        chunk_red = {}

        # chunk 0, both l-halves
        chunk_ps[0] = alloc_ps(0)
        emit_chunk_j(*P_CHUNKS[0], 0, chunk_ps[0][0])
        emit_chunk_j(*P_CHUNKS[0], 1, chunk_ps[0][1])

        # norm matmuls: f16 ones channel-sum per l-half, placed after chunk
        # 0 so the fp16 tree has a wide completion margin.
        nm = [psum.tile([1, NL], f32, tag="ps", name=f"nm_{j}") for j in range(2)]
        nc.tensor.matmul(nm[0][:], ones_col[0:C, :], normc_f[0:C, :], start=True, stop=True)
        nc.tensor.matmul(nm[1][:], ones_col[C : 2 * C, :], normc_f[C : 2 * C, :], start=True, stop=True)

        nsq = nrm.tile([1, 2 * NL], f32)
        nc.vector.tensor_scalar_max(nsq[:, 0:NL], nm[0][:], 1e-8)
        nc.vector.tensor_scalar_max(nsq[:, NL : 2 * NL], nm[1][:], 1e-8)
        nrm_s = nrm.tile([1, 2 * NL], f32)
        nc.scalar.activation(nrm_s[:], nsq[:], SQRT)
        inv_s = nrm.tile([1, 2 * NL], f32)
        nc.vector.reciprocal(inv_s[:], nrm_s[:])
        inv16 = nrm.tile([1, 2 * NL], f16)
        nc.vector.tensor_copy(inv16[:], inv_s[:])

        # chunk 1 first half
        chunk_ps[1] = alloc_ps(1)
        emit_chunk_j(*P_CHUNKS[1], 0, chunk_ps[1][0])

        # rank-1 broadcast of 1/norm to all 128 partitions, between chunk
        # 1's halves (inv16 is ready by the time the PE arrives here).
        inv_bc = nrm.tile([128, 2 * NL], f32)
        for j in range(2):
            ip = psum.tile([128, NL], f32, tag="ps", name=f"ip_{j}")
            nc.tensor.matmul(ip[:], ones_row[:], inv16[:, NL * j : NL * (j + 1)], start=True, stop=True)
            nc.vector.tensor_copy(inv_bc[:, NL * j : NL * (j + 1)], ip[:])

        emit_chunk_j(*P_CHUNKS[1], 1, chunk_ps[1][1])

        chunk_red[0] = emit_epi_j0(0, P_CHUNKS[0][1], chunk_ps[0])
        chunk_ps[2] = alloc_ps(2)
        emit_chunk_j(*P_CHUNKS[2], 0, chunk_ps[2][0])
        emit_epi_j1(0, P_CHUNKS[0][1], chunk_ps[0], chunk_red[0])
        nc.gpsimd.dma_start(out_d[:, 0:1], red_all[:, 0:1])
        chunk_red[1] = emit_epi_j0(1, P_CHUNKS[1][1], chunk_ps[1])
        emit_chunk_j(*P_CHUNKS[2], 1, chunk_ps[2][1])
        emit_epi_j1(1, P_CHUNKS[1][1], chunk_ps[1], chunk_red[1])
        nc.gpsimd.dma_start(out_d[:, 1:2], red_all[:, 1:2])
        chunk_ps[3] = alloc_ps(3)
        emit_chunk_j(*P_CHUNKS[3], 0, chunk_ps[3][0])
        chunk_red[2] = emit_epi_j0(2, P_CHUNKS[2][1], chunk_ps[2])
        emit_chunk_j(*P_CHUNKS[3], 1, chunk_ps[3][1])
        emit_epi_j1(2, P_CHUNKS[2][1], chunk_ps[2], chunk_red[2])
        nc.gpsimd.dma_start(out_d[:, 2:3], red_all[:, 2:3])
        chunk_red[3] = emit_epi_j0(3, P_CHUNKS[3][1], chunk_ps[3])
        emit_epi_j1(3, P_CHUNKS[3][1], chunk_ps[3], chunk_red[3])
        nc.gpsimd.dma_start(out_d[:, 3:4], red_all[:, 3:4])

    nc.compile()
    return nc


def _get_program():
    global _PROGRAM
    if _PROGRAM is None:
        _PROGRAM = _build_program()
    return _PROGRAM


F8 = ml_dtypes.float8_e4m3
BF16 = ml_dtypes.bfloat16


def _pad32(t):
    """[128, R, 30] -> [128, R*32] f32, rows padded 30->32 with 0."""
    padded = np.zeros((128, IM2_ROWS, WPAD), np.float32)
    padded[:, :, :WP] = t
    return padded.reshape(128, -1)


def make_in_maps(im1: np.ndarray, im2: np.ndarray):
    im1 = np.asarray(im1, dtype=np.float32)
    im2 = np.asarray(im2, dtype=np.float32)
    in_maps = []
    for b in range(B):
        i1 = im1[b]
        i1pad = np.concatenate([i1, np.zeros((C, 1, W), np.float32)], axis=1)
        i1s = i1pad[:, 1 : H + 1, :]  # rows shifted down one, zero row at end
        i1x = np.concatenate(
            [i1[:, :, 1:], np.zeros((C, H, 1), np.float32)], axis=2
        )  # cols shifted left one (hi-partition half of the raw tensors)
        raw = np.concatenate([i1, i1x], axis=0).reshape(128, -1)
        m5 = np.concatenate([i1[:, :, 10:40], i1s[:, :, 10:40]], axis=0).reshape(128, -1)
        raw8 = np.ascontiguousarray(raw.astype(F8))
        raw16 = np.ascontiguousarray(raw.astype(BF16))
        m58 = np.ascontiguousarray(m5.astype(F8))
        m516 = np.ascontiguousarray(m5.astype(BF16))
        im1n = np.ascontiguousarray(
            np.concatenate(
                [i1[:, 0:IM2_ROWS, :], i1[:, HALF : HALF + IM2_ROWS, :]], axis=0
            ).astype(BF16)
        )
        for h in range(2):
            y0 = HALF * h
            i2 = im2[b][:, y0 : y0 + IM2_ROWS, :]
            i2pad = np.concatenate(
                [im2[b], np.zeros((C, 1, W), np.float32)], axis=1
            )[:, y0 + 1 : y0 + 1 + IM2_ROWS, :]
            wt = []
            for bi, dx in enumerate(DX_BASES):
                if dx < 10:
                    wt.append(
                        _pad32(
                            np.concatenate(
                                [i2[:, :, dx : dx + WP], i2[:, :, dx + 1 : dx + WP + 1]],
                                axis=0,
                            )
                        )
                    )
                else:
                    wt.append(
                        _pad32(np.concatenate([i2[:, :, 10:40], i2pad[:, :, 10:40]], axis=0))
                    )
            m = {
                "raw8": raw8,
                "raw16": raw16,
                "m58": m58,
                "m516": m516,
                "im1n": im1n,
                "w8": np.ascontiguousarray(
                    np.concatenate([wt[i] for i in (0, 1, 2, 5)], axis=1).astype(F8)
                ),
                "w16": np.ascontiguousarray(np.concatenate(wt[2:6], axis=1).astype(BF16)),
            }
            in_maps.append(m)
    return in_maps


def _half_from_cols(cols):
    flat = np.empty((HALF * WPAD,), dtype=np.float32)
    for ci, (p0, M) in enumerate(P_CHUNKS):
        flat[p0 : p0 + M] = cols[0:M, ci]
    return flat.reshape(HALF, WPAD)[:, :WP]


def assemble(results):
    out = np.empty((B, 1, HP, WP), dtype=np.float32)
    for b in range(B):
        top = _half_from_cols(results[2 * b]["out"])
        bot = _half_from_cols(results[2 * b + 1]["out"])
        out[b, 0] = np.concatenate([top, bot], axis=0)
    return out


def run(im1: np.ndarray, im2: np.ndarray, trace: bool = False):
    from concourse import bass_utils

    nc = _get_program()
    res = bass_utils.run_bass_kernel_spmd(
        nc, make_in_maps(im1, im2), core_ids=list(range(N_CORES)), trace=trace
    )
    return assemble(res.results), res


def kernel(im1: np.ndarray, im2: np.ndarray) -> np.ndarray:
    out, _ = run(np.asarray(im1), np.asarray(im2))
    return out
